# revision 10
# baseline (speedup 1.0000x reference)
"""Trainium2 Bass kernel for CrossTokenMLPAggregator (top-k masked attention aggregation).

Computes, for full inputs
    mlp_hidden   [B=2, T=2048, H=1024] f32
    attn_weights [B=2, Hh=16, T=2048, T=2048] f32
the reference:
    W = attn_weights.mean(axis=1)              # [B, T, T]
    keep top-8 per query row, renormalize kept mass to sum 1
    out = einsum('bts,bsh->bth', W_sparse, mlp_hidden)

Sharding: 8 cores, each owns 512 query rows (core c -> batch c//4,
query rows (c%4)*512 ...). Each core streams its [16, 512, 2048] slice of
attn_weights (the 512 MiB input dominates; the split is exact), sums the
heads sequentially (bit-exact with the reference's mean accumulation
order, so the top-8 selection matches exactly), finds the top-8 with the
DVE max8 instruction, masks with (W >= v8)*W in one scalar_tensor_tensor
op, transposes the masked rows on the TensorEngine and contracts in bf16
(fp32 PSUM accumulate) against the bf16 mlp_hidden slice resident in
SBUF. Renormalization (1/kept fp32 mass) rides the PSUM->SBUF eviction
on the ScalarEngine; outputs are stored bf16 and upcast to f32 on the
host (~0.1% extra rounding on a 2e-2 budget).

Schedule notes (profiled: the stream sustains ~420 GB/s with 2 MiB
DMAs, 1 MiB only ~340; DVE tensor_tensor f32 runs ~114 G elem/s, so a
full-width add is ~2.3 us and 15 of them + mask + max8 would saturate
DVE at ~99% of the 40 us/tile stream window):
- EVERY stream DMA is a full 2 MiB head pair; the first pair is fused
  into acc with one two-operand add (acc = h0 + h1, identical
  arithmetic order), so there are exactly 8 pair DMAs per tile and no
  sub-2 MiB transfers anywhere in the attn stream.
- The head-add chain is COLUMN-SPLIT across engines: DVE owns s-columns
  [0, 1024), GpSimd (otherwise idle) owns [1024, 2048). Each column's
  adds still run in ascending head order on one engine, so the sum is
  bit-exact with the reference; the per-engine load drops to ~17 us /
  ~13 us per 40 us tile, and the post-stream serial drain (the last
  pair's adds) halves.
- The top-8 mask is likewise split: DVE masks the left half, GpSimd the
  right half, partial kept-sums summed after. Transposes of each half
  start as soon as that half's mask lands; PSUM->SBUF evictions of the
  transposed chunks alternate Scalar/Vector so they pipeline.
- Per-tile epilogues are software-pipelined one tile late, emitted
  mid-way through the next tile's add chain.
- HAM power management gates the PE to half clock in ~30 us windows
  between ~10 us full-clock grants; grants are ACTIVITY-TRIGGERED with
  ~2-4 us latency (observed across runs: each grant aligns with an
  epilogue matmul burst). The DMA stream and DVE/GpSimd are unaffected,
  so mid-stream throttling is invisible - but the tail matmul block
  lands at half clock without help. A dense burst of throwaway
  transposes, dependency-timed to the last pair's adds, triggers the
  grant so the 10.24 us full-clock window covers the tail matmuls,
  which run nh-major so the first output half stores while the second
  computes.
- The mlp load is staged fp32 on the SCALAR HWDGE ring (16 x 512 KiB)
  and cast to bf16 on the ScalarEngine, keeping the Sync ring pure
  attn; output DMAs also issue from the Scalar ring.
"""

import numpy as np

B, T, H, Hh, K = 2, 2048, 1024, 16, 8
NCORES = 8
QPC = (B * T) // NCORES          # 512 query rows per core
P = 128                          # partitions
TQ_TILES = QPC // P              # 4 tiles of 128 query rows
S_CHUNKS = T // P                # 16 contraction chunks
HF = T // 2                      # DVE/GpSimd column split point
EPS_SUM = np.float32(1e-8) * np.float32(16.0)  # EPS in head-sum domain

_compiled = {}


def _build_nc():
    import concourse.bass as bass
    import concourse.bacc as bacc
    import concourse.mybir as mybir
    import concourse.tile as tile
    from concourse import masks

    f32 = mybir.dt.float32
    bf16 = mybir.dt.bfloat16
    nc = bacc.Bacc(
        "TRN2",
        target_bir_lowering=False,
        debug=False,
        enable_asserts=False,
        num_devices=NCORES,
    )
    attn = nc.dram_tensor("attn", [Hh, QPC, T], f32, kind="ExternalInput").ap()
    mlp = nc.dram_tensor("mlp", [T, H], f32, kind="ExternalInput").ap()
    out = nc.dram_tensor("out", [QPC, H], bf16, kind="ExternalOutput").ap()

    with tile.TileContext(nc) as tc:
        with (
            tc.tile_pool(name="persist", bufs=1) as persist,
            tc.tile_pool(name="heads", bufs=6) as heads,
            tc.tile_pool(name="acc", bufs=3) as accp,
            tc.tile_pool(name="stage", bufs=3) as stagep,
            tc.tile_pool(name="wm", bufs=1) as wmp,
            tc.tile_pool(name="wmt", bufs=1) as wmtp,
            tc.tile_pool(name="small", bufs=2) as small,
            tc.tile_pool(name="outsb", bufs=2) as outsbp,
            tc.tile_pool(name="tp_psum", bufs=2, space="PSUM") as tp_psum,
            tc.tile_pool(name="mm_psum", bufs=2, space="PSUM") as mm_psum,
        ):
            mlp_sb = persist.tile([P, S_CHUNKS, H], bf16)
            ident = persist.tile([P, P], f32)
            masks.make_identity(nc, ident[:])
            ident_bf = persist.tile([P, P], bf16)
            masks.make_identity(nc, ident_bf[:])

            LEFT = slice(0, HF)
            RIGHT = slice(HF, T)

            def load_mlp_all():
                # fp32 chunks staged via the Scalar HWDGE ring, cast bf16
                # on ScalarE (3 stage bufs pipeline DMA vs cast)
                for c in range(S_CHUNKS):
                    st = stagep.tile([P, H], f32, tag="st")
                    nc.scalar.dma_start(out=st, in_=mlp[c * P : (c + 1) * P, :])
                    nc.scalar.copy(mlp_sb[:, c, :], st)

            def transpose_half(wm, wmt, half):
                # 8 chunk transposes per mask half; evictions alternate
                # Scalar/Vector so they pipeline instead of serializing
                # on one engine
                for g in range(2 * half, 2 * half + 2):
                    pt = tp_psum.tile([P, 4 * P], bf16, tag="ptb")
                    for j in range(4):
                        c = 4 * g + j
                        nc.tensor.transpose(
                            pt[:, j * P : (j + 1) * P],
                            wm[:, c * P : (c + 1) * P],
                            ident_bf[:],
                        )
                    if g % 2 == 0:
                        nc.scalar.copy(wmt[:, 4 * g : 4 * g + 4, :], pt[:])
                    else:
                        nc.vector.tensor_copy(
                            out=wmt[:, 4 * g : 4 * g + 4, :], in_=pt[:]
                        )

            def epilogue(acc, mx, q, tail=False):
                # mask with the row's 8th-largest, in two s-halves so the
                # TensorEngine starts transposing ~1.3us after the top-8
                # lands (GpSimd cannot take this op: scalar_tensor_tensor
                # with a per-partition scalar ptr fails the Pool engine
                # check, so both halves run on DVE).
                v8 = mx[:, K - 1 : K]
                wm = wmp.tile([P, T], bf16, tag="wm")
                wmt = wmtp.tile([P, S_CHUNKS, P], bf16, tag="wmt")
                stot = small.tile([P, 1], f32, tag="stot")
                ssum = small.tile([P, 2], f32, tag="ssum")
                nc.vector.scalar_tensor_tensor(
                    out=wm[:, LEFT],
                    in0=acc[:, LEFT],
                    scalar=v8,
                    in1=acc[:, LEFT],
                    op0=mybir.AluOpType.is_ge,
                    op1=mybir.AluOpType.mult,
                    accum_out=ssum[:, 0:1],
                )
                nc.vector.scalar_tensor_tensor(
                    out=wm[:, RIGHT],
                    in0=acc[:, RIGHT],
                    scalar=v8,
                    in1=acc[:, RIGHT],
                    op0=mybir.AluOpType.is_ge,
                    op1=mybir.AluOpType.mult,
                    accum_out=ssum[:, 1:2],
                )
                transpose_half(wm, wmt, 0)
                transpose_half(wm, wmt, 1)
                nc.vector.tensor_add(out=stot, in0=ssum[:, 0:1], in1=ssum[:, 1:2])
                nc.vector.tensor_scalar_max(stot, stot, float(EPS_SUM))
                rcp = small.tile([P, 1], f32, tag="rcp")
                nc.vector.reciprocal(rcp, stot)

                ps0 = mm_psum.tile([P, 512], f32, tag="ps0")
                ps1 = mm_psum.tile([P, 512], f32, tag="ps1")
                ps = [ps0, ps1]
                osb0 = outsbp.tile([P, 512], bf16, tag="osb0")
                osb1 = outsbp.tile([P, 512], bf16, tag="osb1")
                osb = [osb0, osb1]

                def evict(nh):
                    nsl = slice(nh * 512, (nh + 1) * 512)
                    nc.scalar.activation(
                        out=osb[nh][:, :],
                        in_=ps[nh][:, :],
                        func=mybir.ActivationFunctionType.Copy,
                        scale=rcp[:, :],
                    )
                    nc.scalar.dma_start(out=out[q, nsl], in_=osb[nh][:, :])

                if tail:
                    # nh-major: first output half evicts + stores while the
                    # second half's matmuls run (separate PSUM tiles per half
                    # so the second half's start isn't fenced on the first
                    # half's eviction read)
                    for nh in range(H // 512):
                        nsl = slice(nh * 512, (nh + 1) * 512)
                        for c in range(S_CHUNKS):
                            nc.tensor.matmul(
                                ps[nh][:, :],
                                lhsT=wmt[:, c, :],
                                rhs=mlp_sb[:, c, nsl],
                                start=(c == 0),
                                stop=(c == S_CHUNKS - 1),
                            )
                        evict(nh)
                else:
                    for c in range(S_CHUNKS):
                        for nh in range(H // 512):
                            nsl = slice(nh * 512, (nh + 1) * 512)
                            nc.tensor.matmul(
                                ps[nh][:, :],
                                lhsT=wmt[:, c, :],
                                rhs=mlp_sb[:, c, nsl],
                                start=(c == 0),
                                stop=(c == S_CHUNKS - 1),
                            )
                    for nh in range(H // 512):
                        evict(nh)

            def top8(acc):
                # split max8: the left-half scan only waits on DVE's own
                # add chain, overlapping GpSimd's right-half drain; the
                # 8 largest of the two half-top8s are exactly the row's
                # top-8 (multiset semantics survive the merge)
                mxh = small.tile([P, 2 * K], f32, tag="mxh")
                nc.vector.max(out=mxh[:, 0:K], in_=acc[:, LEFT])
                nc.vector.max(out=mxh[:, K : 2 * K], in_=acc[:, RIGHT])
                mx = small.tile([P, K], f32, tag="mx")
                nc.vector.max(out=mx, in_=mxh)
                return mx

            pending = []  # deferred (acc, mx, q) epilogues

            def run_pending():
                if pending:
                    epilogue(*pending.pop())

            def accumulate(acc, q, warm_pe=False, mid_cb=None):
                # Sequential h order per column: bit-exact with the
                # reference mean's accumulation order, so top-8 selection
                # matches. Heads arrive PAIRED in 2 MiB DMAs; DVE adds
                # the left s-half of each head, GpSimd the right half
                # (independent per-column chains, still ascending h).
                first = True
                for hp in range(0, Hh, 2):
                    ht2 = heads.tile([P, 2, T], f32, tag="ht2")
                    nc.sync.dma_start(
                        out=ht2,
                        in_=attn[hp : hp + 2, q, :].transpose([1, 0, 2]),
                    )
                    if first:
                        nc.vector.tensor_add(
                            out=acc[:, LEFT], in0=ht2[:, 0, LEFT], in1=ht2[:, 1, LEFT]
                        )
                        nc.gpsimd.tensor_add(
                            out=acc[:, RIGHT], in0=ht2[:, 0, RIGHT], in1=ht2[:, 1, RIGHT]
                        )
                        first = False
                    else:
                        for j in range(2):
                            nc.vector.tensor_add(
                                out=acc[:, LEFT], in0=acc[:, LEFT], in1=ht2[:, j, LEFT]
                            )
                            nc.gpsimd.tensor_add(
                                out=acc[:, RIGHT], in0=acc[:, RIGHT], in1=ht2[:, j, RIGHT]
                            )
                    if warm_pe and hp >= 6:
                        # keep a trickle of PE activity through the last
                        # tile's stream (dep = the pair's DMA, so they
                        # space out with the stream)
                        pt = tp_psum.tile([P, 4 * P], f32, tag="pt")
                        nc.tensor.transpose(pt[:, 0:P], ht2[:, 0, 0:P], ident[:])
                    if mid_cb is not None and hp == 6:
                        # pipelined previous-tile epilogue, emitted
                        # mid-chain so its DVE/GpSimd ops don't delay the
                        # tail drain
                        mid_cb()

            load_mlp_all()

            for t in range(TQ_TILES - 1):
                q = slice(t * P, (t + 1) * P)
                acc = accp.tile([P, T], f32, tag="acc")
                accumulate(acc, q)
                mx = top8(acc)
                run_pending()
                pending.append((acc, mx, q))

            # last tile: identical streaming; the only specialization is
            # the HAM-grant trigger burst and the nh-major tail epilogue.
            t = TQ_TILES - 1
            q = slice(t * P, (t + 1) * P)
            acc = accp.tile([P, T], f32, tag="acc")
            accumulate(acc, q, warm_pe=True, mid_cb=run_pending)
            # dense PE burst dep'd on the final left-half add: triggers
            # the HAM full-clock grant (~2-4us latency, 10.24us window)
            # so it covers the tail's transpose + matmul block
            for w in range(12):
                ptw = tp_psum.tile([P, 4 * P], f32, tag="pt")
                nc.tensor.transpose(ptw[:, 0:P], acc[:, 0:P], ident[:])
            mx = top8(acc)
            epilogue(acc, mx, q, tail=True)

    nc.compile()
    return nc


def _get_nc():
    if "nc" not in _compiled:
        _compiled["nc"] = _build_nc()
    return _compiled["nc"]


def kernel(mlp_hidden: np.ndarray, attn_weights: np.ndarray) -> np.ndarray:
    from concourse.bass_utils import run_bass_kernel_spmd

    mlp_hidden = np.ascontiguousarray(mlp_hidden, dtype=np.float32)
    attn_weights = np.ascontiguousarray(attn_weights, dtype=np.float32)
    assert mlp_hidden.shape == (B, T, H)
    assert attn_weights.shape == (B, Hh, T, T)

    nc = _get_nc()
    in_maps = []
    for c in range(NCORES):
        b = c // (NCORES // B)
        q0 = (c % (NCORES // B)) * QPC
        in_maps.append(
            {
                "attn": np.ascontiguousarray(attn_weights[b, :, q0 : q0 + QPC, :]),
                "mlp": mlp_hidden[b],
            }
        )
    res = run_bass_kernel_spmd(nc, in_maps, list(range(NCORES)))
    out = np.empty((B, T, H), dtype=np.float32)
    for c in range(NCORES):
        b = c // (NCORES // B)
        q0 = (c % (NCORES // B)) * QPC
        out[b, q0 : q0 + QPC] = res.results[c]["out"].astype(np.float32)
    return out


# revision 12
# speedup vs baseline: 1.3535x; 1.3535x over previous
"""Trainium2 Bass kernel for CrossTokenMLPAggregator (top-k masked attention aggregation).

Computes, for full inputs
    mlp_hidden   [B=2, T=2048, H=1024] f32
    attn_weights [B=2, Hh=16, T=2048, T=2048] f32
the reference:
    W = attn_weights.mean(axis=1)              # [B, T, T]
    keep top-8 per query row, renormalize kept mass to sum 1
    out = einsum('bts,bsh->bth', W_sparse, mlp_hidden)

Sharding: 8 cores, each owns 512 query rows (core c -> batch c//4,
query rows (c%4)*512 ...). Each core streams its [16, 512, 2048] slice of
attn_weights (the 512 MiB input dominates; the split is exact), sums the
heads sequentially (bit-exact with the reference's mean accumulation
order, so the top-8 selection matches exactly), finds the top-8 with the
DVE max8 instruction, masks with (W >= v8)*W in one scalar_tensor_tensor
op, transposes the masked rows on the TensorEngine and contracts in bf16
(fp32 PSUM accumulate) against the bf16 mlp_hidden slice resident in
SBUF. Renormalization (1/kept fp32 mass) rides the PSUM->SBUF eviction
on the ScalarEngine; outputs are stored bf16 and upcast to f32 on the
host (~0.1% extra rounding on a 2e-2 budget).

Schedule notes (profiled: the stream sustains ~420 GB/s with 2 MiB
DMAs, 1 MiB only ~340; DVE tensor_tensor f32 runs ~114 G elem/s, so a
full-width add is ~2.3 us and 15 of them + mask + max8 would saturate
DVE at ~99% of the 40 us/tile stream window):
- EVERY stream DMA is a full 2 MiB head pair; the first pair is fused
  into acc with one two-operand add (acc = h0 + h1, identical
  arithmetic order), so there are exactly 8 pair DMAs per tile and no
  sub-2 MiB transfers anywhere in the attn stream.
- The head-add chain must stay entirely on DVE: offloading the right
  s-half to GpSimd knocks BOTH engines off their SBUF fast path
  (measured: DVE adds 2.3us -> 3.2us, GpSimd 3.8us per half-add, kernel
  +77us), and GpSimd also rejects scalar_tensor_tensor with a
  per-partition scalar ptr (Pool engine check). DVE runs at ~99% of the
  40us/tile stream window - by design, it is the second roofline.
- The last tile's stream tail is shaped for the DVE drain: h14 arrives
  as one 1 MiB DMA, h15 as two 512 KiB halves added at half width with
  a hierarchical max8 (two half-row max8s merged - exact under multiset
  semantics), so only ~3.6us of serial DVE separates the last byte from
  the mask instead of ~7us.
- The top-8 mask runs in two s-halves so the TensorEngine starts
  transposing ~1.3us after v8 lands; PSUM->SBUF evictions of the
  transposed chunks alternate Scalar/Vector in the tail (DVE is free
  there) and all go to Scalar mid-stream.
- Per-tile epilogues are software-pipelined one tile late, emitted
  mid-way through the next tile's add chain.
- HAM power management gates the PE to half clock in ~30 us windows
  between ~10 us full-clock grants; grants are ACTIVITY-TRIGGERED with
  ~2-4 us latency (observed across runs: each grant aligns with an
  epilogue matmul burst). The DMA stream and DVE/GpSimd are unaffected,
  so mid-stream throttling is invisible - but the tail matmul block
  lands at half clock without help. A dense burst of throwaway
  transposes, dependency-timed to the last pair's adds, triggers the
  grant so the 10.24 us full-clock window covers the tail matmuls,
  which run nh-major so the first output half stores while the second
  computes.
- The mlp load is staged fp32 on the SCALAR HWDGE ring (16 x 512 KiB)
  and cast to bf16 on the ScalarEngine, keeping the Sync ring pure
  attn; output DMAs also issue from the Scalar ring.
"""

import numpy as np

B, T, H, Hh, K = 2, 2048, 1024, 16, 8
NCORES = 8
QPC = (B * T) // NCORES          # 512 query rows per core
P = 128                          # partitions
TQ_TILES = QPC // P              # 4 tiles of 128 query rows
S_CHUNKS = T // P                # 16 contraction chunks
HF = T // 2                      # DVE/GpSimd column split point
EPS_SUM = np.float32(1e-8) * np.float32(16.0)  # EPS in head-sum domain

_compiled = {}


def _build_nc():
    import concourse.bass as bass
    import concourse.bacc as bacc
    import concourse.mybir as mybir
    import concourse.tile as tile
    from concourse import masks

    f32 = mybir.dt.float32
    bf16 = mybir.dt.bfloat16
    nc = bacc.Bacc(
        "TRN2",
        target_bir_lowering=False,
        debug=False,
        enable_asserts=False,
        num_devices=NCORES,
    )
    attn = nc.dram_tensor("attn", [Hh, QPC, T], f32, kind="ExternalInput").ap()
    mlp = nc.dram_tensor("mlp", [T, H], f32, kind="ExternalInput").ap()
    out = nc.dram_tensor("out", [QPC, H], bf16, kind="ExternalOutput").ap()

    with tile.TileContext(nc) as tc:
        with (
            tc.tile_pool(name="persist", bufs=1) as persist,
            tc.tile_pool(name="heads", bufs=6) as heads,
            tc.tile_pool(name="heads1", bufs=1) as heads1,
            tc.tile_pool(name="acc", bufs=3) as accp,
            tc.tile_pool(name="stage", bufs=3) as stagep,
            tc.tile_pool(name="wm", bufs=1) as wmp,
            tc.tile_pool(name="wmt", bufs=1) as wmtp,
            tc.tile_pool(name="small", bufs=2) as small,
            tc.tile_pool(name="outsb", bufs=2) as outsbp,
            tc.tile_pool(name="tp_psum", bufs=2, space="PSUM") as tp_psum,
            tc.tile_pool(name="mm_psum", bufs=2, space="PSUM") as mm_psum,
        ):
            mlp_sb = persist.tile([P, S_CHUNKS, H], bf16)
            ident = persist.tile([P, P], f32)
            masks.make_identity(nc, ident[:])
            ident_bf = persist.tile([P, P], bf16)
            masks.make_identity(nc, ident_bf[:])

            LEFT = slice(0, HF)
            RIGHT = slice(HF, T)

            def load_mlp_all():
                # fp32 chunks staged via the Scalar HWDGE ring, cast bf16
                # on ScalarE (3 stage bufs pipeline DMA vs cast)
                for c in range(S_CHUNKS):
                    st = stagep.tile([P, H], f32, tag="st")
                    nc.scalar.dma_start(out=st, in_=mlp[c * P : (c + 1) * P, :])
                    nc.scalar.copy(mlp_sb[:, c, :], st)

            def transpose_half(wm, wmt, half, tail=False):
                # 8 chunk transposes per mask half; in the tail (DVE is
                # otherwise done) evictions alternate Scalar/Vector so
                # they pipeline instead of serializing on one engine;
                # mid-stream they all go to Scalar since DVE runs at
                # ~99% on the add chain
                for g in range(2 * half, 2 * half + 2):
                    pt = tp_psum.tile([P, 4 * P], bf16, tag="ptb")
                    for j in range(4):
                        c = 4 * g + j
                        nc.tensor.transpose(
                            pt[:, j * P : (j + 1) * P],
                            wm[:, c * P : (c + 1) * P],
                            ident_bf[:],
                        )
                    if tail and g % 2 == 1:
                        nc.vector.tensor_copy(
                            out=wmt[:, 4 * g : 4 * g + 4, :], in_=pt[:]
                        )
                    else:
                        nc.scalar.copy(wmt[:, 4 * g : 4 * g + 4, :], pt[:])

            def epilogue(acc, mx, q, tail=False):
                # mask with the row's 8th-largest, in two s-halves so the
                # TensorEngine starts transposing ~1.3us after the top-8
                # lands (GpSimd cannot take this op: scalar_tensor_tensor
                # with a per-partition scalar ptr fails the Pool engine
                # check, so both halves run on DVE).
                v8 = mx[:, K - 1 : K]
                wm = wmp.tile([P, T], bf16, tag="wm")
                wmt = wmtp.tile([P, S_CHUNKS, P], bf16, tag="wmt")
                stot = small.tile([P, 1], f32, tag="stot")
                ssum = small.tile([P, 2], f32, tag="ssum")
                nc.vector.scalar_tensor_tensor(
                    out=wm[:, LEFT],
                    in0=acc[:, LEFT],
                    scalar=v8,
                    in1=acc[:, LEFT],
                    op0=mybir.AluOpType.is_ge,
                    op1=mybir.AluOpType.mult,
                    accum_out=ssum[:, 0:1],
                )
                nc.vector.scalar_tensor_tensor(
                    out=wm[:, RIGHT],
                    in0=acc[:, RIGHT],
                    scalar=v8,
                    in1=acc[:, RIGHT],
                    op0=mybir.AluOpType.is_ge,
                    op1=mybir.AluOpType.mult,
                    accum_out=ssum[:, 1:2],
                )
                transpose_half(wm, wmt, 0, tail=tail)
                transpose_half(wm, wmt, 1, tail=tail)
                nc.vector.tensor_add(out=stot, in0=ssum[:, 0:1], in1=ssum[:, 1:2])
                nc.vector.tensor_scalar_max(stot, stot, float(EPS_SUM))
                rcp = small.tile([P, 1], f32, tag="rcp")
                nc.vector.reciprocal(rcp, stot)

                ps0 = mm_psum.tile([P, 512], f32, tag="ps0")
                ps1 = mm_psum.tile([P, 512], f32, tag="ps1")
                ps = [ps0, ps1]
                osb0 = outsbp.tile([P, 512], bf16, tag="osb0")
                osb1 = outsbp.tile([P, 512], bf16, tag="osb1")
                osb = [osb0, osb1]

                def evict(nh):
                    nsl = slice(nh * 512, (nh + 1) * 512)
                    nc.scalar.activation(
                        out=osb[nh][:, :],
                        in_=ps[nh][:, :],
                        func=mybir.ActivationFunctionType.Copy,
                        scale=rcp[:, :],
                    )
                    nc.scalar.dma_start(out=out[q, nsl], in_=osb[nh][:, :])

                if tail:
                    # nh-major: first output half evicts + stores while the
                    # second half's matmuls run (separate PSUM tiles per half
                    # so the second half's start isn't fenced on the first
                    # half's eviction read)
                    for nh in range(H // 512):
                        nsl = slice(nh * 512, (nh + 1) * 512)
                        for c in range(S_CHUNKS):
                            nc.tensor.matmul(
                                ps[nh][:, :],
                                lhsT=wmt[:, c, :],
                                rhs=mlp_sb[:, c, nsl],
                                start=(c == 0),
                                stop=(c == S_CHUNKS - 1),
                            )
                        evict(nh)
                else:
                    for c in range(S_CHUNKS):
                        for nh in range(H // 512):
                            nsl = slice(nh * 512, (nh + 1) * 512)
                            nc.tensor.matmul(
                                ps[nh][:, :],
                                lhsT=wmt[:, c, :],
                                rhs=mlp_sb[:, c, nsl],
                                start=(c == 0),
                                stop=(c == S_CHUNKS - 1),
                            )
                    for nh in range(H // 512):
                        evict(nh)

            pending = []  # deferred (acc, mx, q) epilogues

            def run_pending():
                if pending:
                    epilogue(*pending.pop())

            def accumulate(acc, q, warm_pe=False, mid_cb=None, upto=Hh):
                # Sequential h order on DVE: bit-exact with the reference
                # mean's accumulation order, so top-8 selection matches.
                # Heads arrive PAIRED in 2 MiB DMAs; the first pair is
                # fused into acc with one two-operand add (identical
                # arithmetic order). GpSimd cannot share this work:
                # concurrent DVE+GpSimd tensor ops knock BOTH off their
                # SBUF fast path (measured: DVE 2.3us->3.2us, GpSimd
                # 3.8us per half-add), so everything stays on DVE.
                first = True
                for hp in range(0, upto - 1, 2):
                    ht2 = heads.tile([P, 2, T], f32, tag="ht2")
                    nc.sync.dma_start(
                        out=ht2,
                        in_=attn[hp : hp + 2, q, :].transpose([1, 0, 2]),
                    )
                    if first:
                        nc.vector.tensor_add(
                            out=acc, in0=ht2[:, 0, :], in1=ht2[:, 1, :]
                        )
                        first = False
                    else:
                        for j in range(2):
                            nc.vector.tensor_add(
                                out=acc, in0=acc, in1=ht2[:, j, :]
                            )
                    if warm_pe and hp >= 6:
                        # keep a trickle of PE activity through the last
                        # tile's stream (dep = the pair's DMA, so they
                        # space out with the stream)
                        pt = tp_psum.tile([P, 4 * P], f32, tag="pt")
                        nc.tensor.transpose(pt[:, 0:P], ht2[:, 0, 0:P], ident[:])
                    if mid_cb is not None and hp == 6:
                        # pipelined previous-tile epilogue, emitted
                        # mid-chain so its DVE ops don't delay the tail
                        # drain
                        mid_cb()

            load_mlp_all()

            for t in range(TQ_TILES - 1):
                q = slice(t * P, (t + 1) * P)
                acc = accp.tile([P, T], f32, tag="acc")
                accumulate(acc, q)
                mx = small.tile([P, K], f32, tag="mx")
                nc.vector.max(out=mx, in_=acc)
                run_pending()
                pending.append((acc, mx, q))

            # last tile: identical streaming; the only specialization is
            # the HAM-grant trigger burst and the nh-major tail epilogue.
            t = TQ_TILES - 1
            q = slice(t * P, (t + 1) * P)
            acc = accp.tile([P, T], f32, tag="acc")
            accumulate(acc, q, warm_pe=True, mid_cb=run_pending, upto=Hh - 1)
            # stream tail: h14 as one 1 MiB DMA, h15 as two 512 KiB
            # halves with HALF-width adds and a hierarchical max8, so the
            # post-stream DVE serial chain is half-add + 2 half-max8s +
            # merge (~3.6us) instead of two full adds + full max8 (~7us).
            # Costs ~1.4us of stream rate on the last 2 MiB - net win.
            ht1 = heads1.tile([P, T], f32, tag="ht1")
            nc.sync.dma_start(out=ht1, in_=attn[Hh - 2, q, :])
            nc.vector.tensor_add(out=acc, in0=acc, in1=ht1)
            htl = heads1.tile([P, T], f32, tag="htl")
            nc.sync.dma_start(out=htl[:, LEFT], in_=attn[Hh - 1, q, LEFT])
            nc.sync.dma_start(out=htl[:, RIGHT], in_=attn[Hh - 1, q, RIGHT])
            mxh = small.tile([P, 2 * K], f32, tag="mxh")
            nc.vector.tensor_add(
                out=acc[:, LEFT], in0=acc[:, LEFT], in1=htl[:, LEFT]
            )
            # dense PE burst dep'd on the left-half final add: triggers
            # the HAM full-clock grant (~2-4us latency, 10.24us window)
            # so it covers the tail's transpose + matmul block
            for w in range(12):
                ptw = tp_psum.tile([P, 4 * P], f32, tag="pt")
                nc.tensor.transpose(ptw[:, 0:P], acc[:, 0:P], ident[:])
            nc.vector.max(out=mxh[:, 0:K], in_=acc[:, LEFT])
            nc.vector.tensor_add(
                out=acc[:, RIGHT], in0=acc[:, RIGHT], in1=htl[:, RIGHT]
            )
            nc.vector.max(out=mxh[:, K : 2 * K], in_=acc[:, RIGHT])
            mx = small.tile([P, K], f32, tag="mx")
            nc.vector.max(out=mx, in_=mxh)
            epilogue(acc, mx, q, tail=True)

    nc.compile()
    return nc


def _get_nc():
    if "nc" not in _compiled:
        _compiled["nc"] = _build_nc()
    return _compiled["nc"]


def kernel(mlp_hidden: np.ndarray, attn_weights: np.ndarray) -> np.ndarray:
    from concourse.bass_utils import run_bass_kernel_spmd

    mlp_hidden = np.ascontiguousarray(mlp_hidden, dtype=np.float32)
    attn_weights = np.ascontiguousarray(attn_weights, dtype=np.float32)
    assert mlp_hidden.shape == (B, T, H)
    assert attn_weights.shape == (B, Hh, T, T)

    nc = _get_nc()
    in_maps = []
    for c in range(NCORES):
        b = c // (NCORES // B)
        q0 = (c % (NCORES // B)) * QPC
        in_maps.append(
            {
                "attn": np.ascontiguousarray(attn_weights[b, :, q0 : q0 + QPC, :]),
                "mlp": mlp_hidden[b],
            }
        )
    res = run_bass_kernel_spmd(nc, in_maps, list(range(NCORES)))
    out = np.empty((B, T, H), dtype=np.float32)
    for c in range(NCORES):
        b = c // (NCORES // B)
        q0 = (c % (NCORES // B)) * QPC
        out[b, q0 : q0 + QPC] = res.results[c]["out"].astype(np.float32)
    return out


# revision 14
# speedup vs baseline: 1.3563x; 1.0021x over previous
"""Trainium2 Bass kernel for CrossTokenMLPAggregator (top-k masked attention aggregation).

Computes, for full inputs
    mlp_hidden   [B=2, T=2048, H=1024] f32
    attn_weights [B=2, Hh=16, T=2048, T=2048] f32
the reference:
    W = attn_weights.mean(axis=1)              # [B, T, T]
    keep top-8 per query row, renormalize kept mass to sum 1
    out = einsum('bts,bsh->bth', W_sparse, mlp_hidden)

Sharding: 8 cores, each owns 512 query rows (core c -> batch c//4,
query rows (c%4)*512 ...). Each core streams its [16, 512, 2048] slice of
attn_weights (the 512 MiB input dominates; the split is exact), sums the
heads sequentially (bit-exact with the reference's mean accumulation
order, so the top-8 selection matches exactly), finds the top-8 with the
DVE max8 instruction, masks with (W >= v8)*W in one scalar_tensor_tensor
op, transposes the masked rows on the TensorEngine and contracts in bf16
(fp32 PSUM accumulate) against the bf16 mlp_hidden slice resident in
SBUF. Renormalization (1/kept fp32 mass) rides the PSUM->SBUF eviction
on the ScalarEngine; outputs are stored bf16 and upcast to f32 on the
host (~0.1% extra rounding on a 2e-2 budget).

Schedule notes (profiled: the stream sustains ~420 GB/s with 2 MiB
DMAs, 1 MiB only ~340; DVE tensor_tensor f32 runs ~114 G elem/s, so a
full-width add is ~2.3 us and 15 of them + mask + max8 would saturate
DVE at ~99% of the 40 us/tile stream window):
- EVERY stream DMA is a full 2 MiB head pair; the first pair is fused
  into acc with one two-operand add (acc = h0 + h1, identical
  arithmetic order), so there are exactly 8 pair DMAs per tile and no
  sub-2 MiB transfers anywhere in the attn stream.
- The head-add chain must stay entirely on DVE: offloading the right
  s-half to GpSimd knocks BOTH engines off their SBUF fast path
  (measured: DVE adds 2.3us -> 3.2us, GpSimd 3.8us per half-add, kernel
  +77us), and GpSimd also rejects scalar_tensor_tensor with a
  per-partition scalar ptr (Pool engine check). DVE runs at ~99% of the
  40us/tile stream window - by design, it is the second roofline.
- The last tile's stream tail is shaped for the DVE drain: h14 arrives
  as one 1 MiB DMA, h15 as two 512 KiB halves added at half width with
  a hierarchical max8 (two half-row max8s merged - exact under multiset
  semantics), so only ~3.6us of serial DVE separates the last byte from
  the mask instead of ~7us.
- The top-8 mask runs in two s-halves so the TensorEngine starts
  transposing ~1.3us after v8 lands; PSUM->SBUF evictions of the
  transposed chunks alternate Scalar/Vector in the tail (DVE is free
  there) and all go to Scalar mid-stream.
- Per-tile epilogues are software-pipelined one tile late, emitted
  mid-way through the next tile's add chain.
- HAM power management gates the PE to half clock in ~30 us windows
  between ~10 us full-clock grants; grants are ACTIVITY-TRIGGERED with
  ~2-4 us latency (observed across runs: each grant aligns with an
  epilogue matmul burst). The DMA stream and DVE/GpSimd are unaffected,
  so mid-stream throttling is invisible - but the tail matmul block
  lands at half clock without help. A dense burst of throwaway
  transposes, dependency-timed to the last pair's adds, triggers the
  grant so the 10.24 us full-clock window covers the tail matmuls,
  which run nh-major so the first output half stores while the second
  computes.
- The mlp load is staged fp32 on the SCALAR HWDGE ring (16 x 512 KiB)
  and cast to bf16 on the ScalarEngine, keeping the Sync ring pure
  attn; output DMAs also issue from the Scalar ring.
"""

import numpy as np

B, T, H, Hh, K = 2, 2048, 1024, 16, 8
NCORES = 8
QPC = (B * T) // NCORES          # 512 query rows per core
P = 128                          # partitions
TQ_TILES = QPC // P              # 4 tiles of 128 query rows
S_CHUNKS = T // P                # 16 contraction chunks
HF = T // 2                      # DVE/GpSimd column split point
EPS_SUM = np.float32(1e-8) * np.float32(16.0)  # EPS in head-sum domain

_compiled = {}


def _build_nc():
    import concourse.bass as bass
    import concourse.bacc as bacc
    import concourse.mybir as mybir
    import concourse.tile as tile
    from concourse import masks

    f32 = mybir.dt.float32
    bf16 = mybir.dt.bfloat16
    nc = bacc.Bacc(
        "TRN2",
        target_bir_lowering=False,
        debug=False,
        enable_asserts=False,
        num_devices=NCORES,
    )
    attn = nc.dram_tensor("attn", [Hh, QPC, T], f32, kind="ExternalInput").ap()
    mlp = nc.dram_tensor("mlp", [T, H], f32, kind="ExternalInput").ap()
    out = nc.dram_tensor("out", [QPC, H], bf16, kind="ExternalOutput").ap()

    with tile.TileContext(nc) as tc:
        with (
            tc.tile_pool(name="persist", bufs=1) as persist,
            tc.tile_pool(name="heads", bufs=6) as heads,
            tc.tile_pool(name="heads1", bufs=1) as heads1,
            tc.tile_pool(name="acc", bufs=3) as accp,
            tc.tile_pool(name="stage", bufs=2) as stagep,
            tc.tile_pool(name="wm", bufs=1) as wmp,
            tc.tile_pool(name="wmt", bufs=1) as wmtp,
            tc.tile_pool(name="small", bufs=2) as small,
            tc.tile_pool(name="outsb", bufs=2) as outsbp,
            tc.tile_pool(name="tp_psum", bufs=2, space="PSUM") as tp_psum,
            tc.tile_pool(name="mm_psum", bufs=2, space="PSUM") as mm_psum,
        ):
            mlp_sb = persist.tile([P, S_CHUNKS, H], bf16)
            ident = persist.tile([P, P], f32)
            masks.make_identity(nc, ident[:])
            ident_bf = persist.tile([P, P], bf16)
            masks.make_identity(nc, ident_bf[:])

            LEFT = slice(0, HF)
            RIGHT = slice(HF, T)

            def load_mlp_all():
                # fp32 chunk PAIRS (1 MiB DMAs - 512 KiB transfers drag
                # the shared SDMA packet round-robin and dipped the attn
                # stream to ~345 GB/s during staging) via the Scalar
                # HWDGE ring, cast bf16 on ScalarE
                mlp2 = mlp.rearrange(
                    "(a b p) h -> a b p h", a=S_CHUNKS // 2, b=2, p=P
                )
                for c2 in range(S_CHUNKS // 2):
                    st = stagep.tile([P, 2, H], f32, tag="st")
                    nc.scalar.dma_start(
                        out=st, in_=mlp2[c2].transpose([1, 0, 2])
                    )
                    nc.scalar.copy(mlp_sb[:, 2 * c2 : 2 * c2 + 2, :], st)

            def transpose_half(wm, wmt, half, tail=False):
                # 8 chunk transposes per mask half; in the tail (DVE is
                # otherwise done) evictions alternate Scalar/Vector so
                # they pipeline instead of serializing on one engine;
                # mid-stream they all go to Scalar since DVE runs at
                # ~99% on the add chain
                for g in range(2 * half, 2 * half + 2):
                    pt = tp_psum.tile([P, 4 * P], bf16, tag="ptb")
                    for j in range(4):
                        c = 4 * g + j
                        nc.tensor.transpose(
                            pt[:, j * P : (j + 1) * P],
                            wm[:, c * P : (c + 1) * P],
                            ident_bf[:],
                        )
                    if tail and g % 2 == 1:
                        nc.vector.tensor_copy(
                            out=wmt[:, 4 * g : 4 * g + 4, :], in_=pt[:]
                        )
                    else:
                        nc.scalar.copy(wmt[:, 4 * g : 4 * g + 4, :], pt[:])

            def epilogue(acc, mx, q, tail=False):
                # mask with the row's 8th-largest, in two s-halves so the
                # TensorEngine starts transposing ~1.3us after the top-8
                # lands (GpSimd cannot take this op: scalar_tensor_tensor
                # with a per-partition scalar ptr fails the Pool engine
                # check, so both halves run on DVE).
                v8 = mx[:, K - 1 : K]
                wm = wmp.tile([P, T], bf16, tag="wm")
                wmt = wmtp.tile([P, S_CHUNKS, P], bf16, tag="wmt")
                stot = small.tile([P, 1], f32, tag="stot")
                ssum = small.tile([P, 2], f32, tag="ssum")
                nc.vector.scalar_tensor_tensor(
                    out=wm[:, LEFT],
                    in0=acc[:, LEFT],
                    scalar=v8,
                    in1=acc[:, LEFT],
                    op0=mybir.AluOpType.is_ge,
                    op1=mybir.AluOpType.mult,
                    accum_out=ssum[:, 0:1],
                )
                nc.vector.scalar_tensor_tensor(
                    out=wm[:, RIGHT],
                    in0=acc[:, RIGHT],
                    scalar=v8,
                    in1=acc[:, RIGHT],
                    op0=mybir.AluOpType.is_ge,
                    op1=mybir.AluOpType.mult,
                    accum_out=ssum[:, 1:2],
                )
                transpose_half(wm, wmt, 0, tail=tail)
                transpose_half(wm, wmt, 1, tail=tail)
                nc.vector.tensor_add(out=stot, in0=ssum[:, 0:1], in1=ssum[:, 1:2])
                nc.vector.tensor_scalar_max(stot, stot, float(EPS_SUM))
                rcp = small.tile([P, 1], f32, tag="rcp")
                nc.vector.reciprocal(rcp, stot)

                ps0 = mm_psum.tile([P, 512], f32, tag="ps0")
                ps1 = mm_psum.tile([P, 512], f32, tag="ps1")
                ps = [ps0, ps1]
                osb0 = outsbp.tile([P, 512], bf16, tag="osb0")
                osb1 = outsbp.tile([P, 512], bf16, tag="osb1")
                osb = [osb0, osb1]

                def evict(nh):
                    nsl = slice(nh * 512, (nh + 1) * 512)
                    nc.scalar.activation(
                        out=osb[nh][:, :],
                        in_=ps[nh][:, :],
                        func=mybir.ActivationFunctionType.Copy,
                        scale=rcp[:, :],
                    )
                    nc.scalar.dma_start(out=out[q, nsl], in_=osb[nh][:, :])

                if tail:
                    # nh-major: first output half evicts + stores while the
                    # second half's matmuls run (separate PSUM tiles per half
                    # so the second half's start isn't fenced on the first
                    # half's eviction read)
                    for nh in range(H // 512):
                        nsl = slice(nh * 512, (nh + 1) * 512)
                        for c in range(S_CHUNKS):
                            nc.tensor.matmul(
                                ps[nh][:, :],
                                lhsT=wmt[:, c, :],
                                rhs=mlp_sb[:, c, nsl],
                                start=(c == 0),
                                stop=(c == S_CHUNKS - 1),
                            )
                        evict(nh)
                else:
                    for c in range(S_CHUNKS):
                        for nh in range(H // 512):
                            nsl = slice(nh * 512, (nh + 1) * 512)
                            nc.tensor.matmul(
                                ps[nh][:, :],
                                lhsT=wmt[:, c, :],
                                rhs=mlp_sb[:, c, nsl],
                                start=(c == 0),
                                stop=(c == S_CHUNKS - 1),
                            )
                    for nh in range(H // 512):
                        evict(nh)

            pending = []  # deferred (acc, mx, q) epilogues

            def run_pending():
                if pending:
                    epilogue(*pending.pop())

            def accumulate(acc, q, warm_pe=False, mid_cb=None, upto=Hh):
                # Sequential h order on DVE: bit-exact with the reference
                # mean's accumulation order, so top-8 selection matches.
                # Heads arrive PAIRED in 2 MiB DMAs; the first pair is
                # fused into acc with one two-operand add (identical
                # arithmetic order). GpSimd cannot share this work:
                # concurrent DVE+GpSimd tensor ops knock BOTH off their
                # SBUF fast path (measured: DVE 2.3us->3.2us, GpSimd
                # 3.8us per half-add), so everything stays on DVE.
                first = True
                for hp in range(0, upto - 1, 2):
                    ht2 = heads.tile([P, 2, T], f32, tag="ht2")
                    nc.sync.dma_start(
                        out=ht2,
                        in_=attn[hp : hp + 2, q, :].transpose([1, 0, 2]),
                    )
                    if first:
                        nc.vector.tensor_add(
                            out=acc, in0=ht2[:, 0, :], in1=ht2[:, 1, :]
                        )
                        first = False
                    else:
                        for j in range(2):
                            nc.vector.tensor_add(
                                out=acc, in0=acc, in1=ht2[:, j, :]
                            )
                    if warm_pe and hp >= 6:
                        # keep a trickle of PE activity through the last
                        # tile's stream (dep = the pair's DMA, so they
                        # space out with the stream); the FINAL pair
                        # (h12/h13) instead fires a dense burst so the
                        # activity-triggered HAM full-clock grant lands
                        # before the tail matmul block
                        n_tp = 24 if hp == Hh - 4 else 1
                        for _ in range(n_tp):
                            pt = tp_psum.tile([P, 4 * P], f32, tag="pt")
                            nc.tensor.transpose(
                                pt[:, 0:P], ht2[:, 0, 0:P], ident[:]
                            )
                    if mid_cb is not None and hp == 6:
                        # pipelined previous-tile epilogue, emitted
                        # mid-chain so its DVE ops don't delay the tail
                        # drain
                        mid_cb()

            load_mlp_all()

            for t in range(TQ_TILES - 1):
                q = slice(t * P, (t + 1) * P)
                acc = accp.tile([P, T], f32, tag="acc")
                accumulate(acc, q)
                mx = small.tile([P, K], f32, tag="mx")
                nc.vector.max(out=mx, in_=acc)
                run_pending()
                pending.append((acc, mx, q))

            # last tile: identical streaming; the only specialization is
            # the HAM-grant trigger burst and the nh-major tail epilogue.
            t = TQ_TILES - 1
            q = slice(t * P, (t + 1) * P)
            acc = accp.tile([P, T], f32, tag="acc")
            accumulate(acc, q, warm_pe=True, mid_cb=run_pending, upto=Hh - 1)
            # stream tail: h14 as one 1 MiB DMA, h15 as two 512 KiB
            # halves with HALF-width adds and a hierarchical max8, so the
            # post-stream DVE serial chain is half-add + 2 half-max8s +
            # merge (~3.6us) instead of two full adds + full max8 (~7us).
            # Costs ~1.4us of stream rate on the last 2 MiB - net win.
            ht1 = heads1.tile([P, T], f32, tag="ht1")
            nc.sync.dma_start(out=ht1, in_=attn[Hh - 2, q, :])
            nc.vector.tensor_add(out=acc, in0=acc, in1=ht1)
            htl = heads1.tile([P, T], f32, tag="htl")
            nc.sync.dma_start(out=htl[:, LEFT], in_=attn[Hh - 1, q, LEFT])
            nc.sync.dma_start(out=htl[:, RIGHT], in_=attn[Hh - 1, q, RIGHT])
            mxh = small.tile([P, 2 * K], f32, tag="mxh")
            nc.vector.tensor_add(
                out=acc[:, LEFT], in0=acc[:, LEFT], in1=htl[:, LEFT]
            )
            # dense PE burst dep'd on the left-half final add: triggers
            # the HAM full-clock grant (~2-4us latency, 10.24us window)
            # so it covers the tail's transpose + matmul block
            for w in range(12):
                ptw = tp_psum.tile([P, 4 * P], f32, tag="pt")
                nc.tensor.transpose(ptw[:, 0:P], acc[:, 0:P], ident[:])
            nc.vector.max(out=mxh[:, 0:K], in_=acc[:, LEFT])
            nc.vector.tensor_add(
                out=acc[:, RIGHT], in0=acc[:, RIGHT], in1=htl[:, RIGHT]
            )
            nc.vector.max(out=mxh[:, K : 2 * K], in_=acc[:, RIGHT])
            mx = small.tile([P, K], f32, tag="mx")
            nc.vector.max(out=mx, in_=mxh)
            epilogue(acc, mx, q, tail=True)

    nc.compile()
    return nc


def _get_nc():
    if "nc" not in _compiled:
        _compiled["nc"] = _build_nc()
    return _compiled["nc"]


def kernel(mlp_hidden: np.ndarray, attn_weights: np.ndarray) -> np.ndarray:
    from concourse.bass_utils import run_bass_kernel_spmd

    mlp_hidden = np.ascontiguousarray(mlp_hidden, dtype=np.float32)
    attn_weights = np.ascontiguousarray(attn_weights, dtype=np.float32)
    assert mlp_hidden.shape == (B, T, H)
    assert attn_weights.shape == (B, Hh, T, T)

    nc = _get_nc()
    in_maps = []
    for c in range(NCORES):
        b = c // (NCORES // B)
        q0 = (c % (NCORES // B)) * QPC
        in_maps.append(
            {
                "attn": np.ascontiguousarray(attn_weights[b, :, q0 : q0 + QPC, :]),
                "mlp": mlp_hidden[b],
            }
        )
    res = run_bass_kernel_spmd(nc, in_maps, list(range(NCORES)))
    out = np.empty((B, T, H), dtype=np.float32)
    for c in range(NCORES):
        b = c // (NCORES // B)
        q0 = (c % (NCORES // B)) * QPC
        out[b, q0 : q0 + QPC] = res.results[c]["out"].astype(np.float32)
    return out


# revision 15
# speedup vs baseline: 1.3580x; 1.0012x over previous
"""Trainium2 Bass kernel for CrossTokenMLPAggregator (top-k masked attention aggregation).

Computes, for full inputs
    mlp_hidden   [B=2, T=2048, H=1024] f32
    attn_weights [B=2, Hh=16, T=2048, T=2048] f32
the reference:
    W = attn_weights.mean(axis=1)              # [B, T, T]
    keep top-8 per query row, renormalize kept mass to sum 1
    out = einsum('bts,bsh->bth', W_sparse, mlp_hidden)

Sharding: 8 cores, each owns 512 query rows (core c -> batch c//4,
query rows (c%4)*512 ...). Each core streams its [16, 512, 2048] slice of
attn_weights (the 512 MiB input dominates; the split is exact), sums the
heads sequentially (bit-exact with the reference's mean accumulation
order, so the top-8 selection matches exactly), finds the top-8 with the
DVE max8 instruction, masks with (W >= v8)*W in one scalar_tensor_tensor
op, transposes the masked rows on the TensorEngine and contracts in bf16
(fp32 PSUM accumulate) against the bf16 mlp_hidden slice resident in
SBUF. Renormalization (1/kept fp32 mass) rides the PSUM->SBUF eviction
on the ScalarEngine; outputs are stored bf16 and upcast to f32 on the
host (~0.1% extra rounding on a 2e-2 budget).

Schedule notes (profiled: the stream sustains ~420 GB/s with 2 MiB
DMAs, 1 MiB only ~340, 512 KiB ~280; DVE tensor_tensor f32 runs
~114 G elem/s, so a full-width add is ~2.3 us and 15 of them + mask +
max8 fill ~99% of the 40 us/tile stream window - DVE is the second
roofline by design):
- EVERY stream DMA is a full 2 MiB head pair; the first pair is fused
  into acc with one two-operand add (acc = h0 + h1, identical
  arithmetic order), so there are exactly 8 pair DMAs per tile and no
  sub-2 MiB transfers anywhere in the attn stream. Shaping the last
  pair smaller to start its adds earlier is a wash: the lost DMA rate
  on sub-1 MiB transfers cancels the earlier add start.
- The head-add chain must stay entirely on DVE: offloading the right
  s-half to GpSimd knocks BOTH engines off their SBUF fast path
  (measured: DVE adds 2.3us -> 3.2us, GpSimd 3.8us per half-add, kernel
  +77us), and GpSimd also rejects scalar_tensor_tensor with a
  per-partition scalar ptr (Pool engine check).
- Mid-stream epilogues mask in two s-halves; the TAIL epilogue masks in
  four s-quarters, each feeding its 4-chunk transpose group, so the
  first matmul starts ~0.65us after v8 instead of ~1.3us. PSUM->SBUF
  evictions of transposed chunks alternate Scalar/Vector in the tail
  (DVE is free there) and all go to Scalar mid-stream. Tail matmuls run
  nh-major so the first output half stores while the second computes.
- Per-tile epilogues are software-pipelined one tile late, emitted
  mid-way through the next tile's add chain. Each mid tile's two output
  evictions share one SBUF tile and one 256 KiB store (two 128 KiB
  stores drag the SDMA packet round-robin).
- HAM power management runs the PE on a ~25% full-clock duty budget in
  3.413us quanta: activity triggers a grant of <= 3 quanta (10.24us),
  the grant is RELEASED as soon as the PE idles, and a cooldown of
  ~3x the granted quanta follows at half clock. Pre-triggering with
  throwaway transposes therefore BACKFIRES - the burst's grant is
  released in the dead gap before the tail matmuls and the forced
  cooldown lands exactly on them (measured: grant 193.9us released
  after 3.4us, cooldown covered the first 16 tail matmuls). So the PE
  is kept silent through the last tile's stream and the tail's own
  transposes trigger the grant naturally.
- The mlp load is staged fp32 in chunk pairs (1 MiB DMAs) on the SCALAR
  HWDGE ring and cast to bf16 on the ScalarEngine, keeping the Sync
  ring pure attn; output DMAs also issue from the Scalar ring.
"""

import numpy as np

B, T, H, Hh, K = 2, 2048, 1024, 16, 8
NCORES = 8
QPC = (B * T) // NCORES          # 512 query rows per core
P = 128                          # partitions
TQ_TILES = QPC // P              # 4 tiles of 128 query rows
S_CHUNKS = T // P                # 16 contraction chunks
HF = T // 2
EPS_SUM = np.float32(1e-8) * np.float32(16.0)  # EPS in head-sum domain

_compiled = {}


def _build_nc():
    import concourse.bass as bass
    import concourse.bacc as bacc
    import concourse.mybir as mybir
    import concourse.tile as tile
    from concourse import masks

    f32 = mybir.dt.float32
    bf16 = mybir.dt.bfloat16
    nc = bacc.Bacc(
        "TRN2",
        target_bir_lowering=False,
        debug=False,
        enable_asserts=False,
        num_devices=NCORES,
    )
    attn = nc.dram_tensor("attn", [Hh, QPC, T], f32, kind="ExternalInput").ap()
    mlp = nc.dram_tensor("mlp", [T, H], f32, kind="ExternalInput").ap()
    out = nc.dram_tensor("out", [QPC, H], bf16, kind="ExternalOutput").ap()

    with tile.TileContext(nc) as tc:
        with (
            tc.tile_pool(name="persist", bufs=1) as persist,
            tc.tile_pool(name="heads", bufs=6) as heads,
            tc.tile_pool(name="acc", bufs=3) as accp,
            tc.tile_pool(name="stage", bufs=2) as stagep,
            tc.tile_pool(name="wm", bufs=1) as wmp,
            tc.tile_pool(name="wmt", bufs=1) as wmtp,
            tc.tile_pool(name="small", bufs=2) as small,
            tc.tile_pool(name="outsb", bufs=2) as outsbp,
            tc.tile_pool(name="tp_psum", bufs=2, space="PSUM") as tp_psum,
            tc.tile_pool(name="mm_psum", bufs=2, space="PSUM") as mm_psum,
        ):
            mlp_sb = persist.tile([P, S_CHUNKS, H], bf16)
            ident_bf = persist.tile([P, P], bf16)
            masks.make_identity(nc, ident_bf[:])

            LEFT = slice(0, HF)
            RIGHT = slice(HF, T)

            def load_mlp_all():
                # fp32 chunk PAIRS (1 MiB DMAs - 512 KiB transfers drag
                # the shared SDMA packet round-robin and dipped the attn
                # stream to ~345 GB/s during staging) via the Scalar
                # HWDGE ring, cast bf16 on ScalarE
                mlp2 = mlp.rearrange(
                    "(a b p) h -> a b p h", a=S_CHUNKS // 2, b=2, p=P
                )
                for c2 in range(S_CHUNKS // 2):
                    st = stagep.tile([P, 2, H], f32, tag="st")
                    nc.scalar.dma_start(
                        out=st, in_=mlp2[c2].transpose([1, 0, 2])
                    )
                    nc.scalar.copy(mlp_sb[:, 2 * c2 : 2 * c2 + 2, :], st)

            def transpose_group(wm, wmt, g, tail=False):
                # transpose one 4-chunk group; in the tail (DVE is
                # otherwise done) evictions alternate Scalar/Vector so
                # they pipeline instead of serializing on one engine;
                # mid-stream they all go to Scalar since DVE runs at
                # ~99% on the add chain
                pt = tp_psum.tile([P, 4 * P], bf16, tag="ptb")
                for j in range(4):
                    c = 4 * g + j
                    nc.tensor.transpose(
                        pt[:, j * P : (j + 1) * P],
                        wm[:, c * P : (c + 1) * P],
                        ident_bf[:],
                    )
                if tail and g % 2 == 1:
                    nc.vector.tensor_copy(
                        out=wmt[:, 4 * g : 4 * g + 4, :], in_=pt[:]
                    )
                else:
                    nc.scalar.copy(wmt[:, 4 * g : 4 * g + 4, :], pt[:])

            def epilogue(acc, mx, q, tail=False):
                # mask with the row's 8th-largest; s-halves mid-stream,
                # s-quarters in the tail so the first transposes (and the
                # matmuls chasing them) start ~0.65us after v8 lands
                v8 = mx[:, K - 1 : K]
                wm = wmp.tile([P, T], bf16, tag="wm")
                wmt = wmtp.tile([P, S_CHUNKS, P], bf16, tag="wmt")
                stot = small.tile([P, 1], f32, tag="stot")
                nparts = 4 if tail else 2
                PW = T // nparts
                ssum = small.tile([P, nparts], f32, tag="ssum4" if tail else "ssum2")
                for p_ in range(nparts):
                    sl = slice(p_ * PW, (p_ + 1) * PW)
                    nc.vector.scalar_tensor_tensor(
                        out=wm[:, sl],
                        in0=acc[:, sl],
                        scalar=v8,
                        in1=acc[:, sl],
                        op0=mybir.AluOpType.is_ge,
                        op1=mybir.AluOpType.mult,
                        accum_out=ssum[:, p_ : p_ + 1],
                    )
                for g in range(4):
                    transpose_group(wm, wmt, g, tail=tail)
                if tail:
                    s2 = small.tile([P, 2], f32, tag="s2")
                    nc.vector.tensor_add(
                        out=s2, in0=ssum[:, 0:2], in1=ssum[:, 2:4]
                    )
                    nc.vector.tensor_add(out=stot, in0=s2[:, 0:1], in1=s2[:, 1:2])
                else:
                    nc.vector.tensor_add(
                        out=stot, in0=ssum[:, 0:1], in1=ssum[:, 1:2]
                    )
                nc.vector.tensor_scalar_max(stot, stot, float(EPS_SUM))
                rcp = small.tile([P, 1], f32, tag="rcp")
                nc.vector.reciprocal(rcp, stot)

                ps0 = mm_psum.tile([P, 512], f32, tag="ps0")
                ps1 = mm_psum.tile([P, 512], f32, tag="ps1")
                ps = [ps0, ps1]
                osb = outsbp.tile([P, H], bf16, tag="osb")

                def evict(nh):
                    nsl = slice(nh * 512, (nh + 1) * 512)
                    nc.scalar.activation(
                        out=osb[:, nsl],
                        in_=ps[nh][:, :],
                        func=mybir.ActivationFunctionType.Copy,
                        scale=rcp[:, :],
                    )

                if tail:
                    # nh-major: first output half evicts + stores while the
                    # second half's matmuls run (separate PSUM tiles per half
                    # so the second half's start isn't fenced on the first
                    # half's eviction read)
                    for nh in range(H // 512):
                        nsl = slice(nh * 512, (nh + 1) * 512)
                        for c in range(S_CHUNKS):
                            nc.tensor.matmul(
                                ps[nh][:, :],
                                lhsT=wmt[:, c, :],
                                rhs=mlp_sb[:, c, nsl],
                                start=(c == 0),
                                stop=(c == S_CHUNKS - 1),
                            )
                        evict(nh)
                        nc.scalar.dma_start(out=out[q, nsl], in_=osb[:, nsl])
                else:
                    for c in range(S_CHUNKS):
                        for nh in range(H // 512):
                            nsl = slice(nh * 512, (nh + 1) * 512)
                            nc.tensor.matmul(
                                ps[nh][:, :],
                                lhsT=wmt[:, c, :],
                                rhs=mlp_sb[:, c, nsl],
                                start=(c == 0),
                                stop=(c == S_CHUNKS - 1),
                            )
                    for nh in range(H // 512):
                        evict(nh)
                    # one 256 KiB store instead of two 128 KiB ones
                    nc.scalar.dma_start(out=out[q, :], in_=osb[:, :])

            pending = []  # deferred (acc, mx, q) epilogues

            def run_pending():
                if pending:
                    epilogue(*pending.pop())

            def accumulate(acc, q, mid_cb=None):
                # Sequential h order on DVE: bit-exact with the reference
                # mean's accumulation order, so top-8 selection matches.
                # Heads arrive PAIRED in 2 MiB DMAs; the first pair is
                # fused into acc with one two-operand add (identical
                # arithmetic order).
                first = True
                for hp in range(0, Hh, 2):
                    ht2 = heads.tile([P, 2, T], f32, tag="ht2")
                    nc.sync.dma_start(
                        out=ht2,
                        in_=attn[hp : hp + 2, q, :].transpose([1, 0, 2]),
                    )
                    if first:
                        nc.vector.tensor_add(
                            out=acc, in0=ht2[:, 0, :], in1=ht2[:, 1, :]
                        )
                        first = False
                    else:
                        for j in range(2):
                            nc.vector.tensor_add(
                                out=acc, in0=acc, in1=ht2[:, j, :]
                            )
                    if mid_cb is not None and hp == 6:
                        # pipelined previous-tile epilogue, emitted
                        # mid-chain so its DVE ops don't delay the tail
                        # drain
                        mid_cb()

            load_mlp_all()

            for t in range(TQ_TILES):
                q = slice(t * P, (t + 1) * P)
                acc = accp.tile([P, T], f32, tag="acc")
                last = t == TQ_TILES - 1
                accumulate(acc, q, mid_cb=run_pending if last else None)
                mx = small.tile([P, K], f32, tag="mx")
                nc.vector.max(out=mx, in_=acc)
                if last:
                    epilogue(acc, mx, q, tail=True)
                else:
                    run_pending()
                    pending.append((acc, mx, q))

    nc.compile()
    return nc


def _get_nc():
    if "nc" not in _compiled:
        _compiled["nc"] = _build_nc()
    return _compiled["nc"]


def kernel(mlp_hidden: np.ndarray, attn_weights: np.ndarray) -> np.ndarray:
    from concourse.bass_utils import run_bass_kernel_spmd

    mlp_hidden = np.ascontiguousarray(mlp_hidden, dtype=np.float32)
    attn_weights = np.ascontiguousarray(attn_weights, dtype=np.float32)
    assert mlp_hidden.shape == (B, T, H)
    assert attn_weights.shape == (B, Hh, T, T)

    nc = _get_nc()
    in_maps = []
    for c in range(NCORES):
        b = c // (NCORES // B)
        q0 = (c % (NCORES // B)) * QPC
        in_maps.append(
            {
                "attn": np.ascontiguousarray(attn_weights[b, :, q0 : q0 + QPC, :]),
                "mlp": mlp_hidden[b],
            }
        )
    res = run_bass_kernel_spmd(nc, in_maps, list(range(NCORES)))
    out = np.empty((B, T, H), dtype=np.float32)
    for c in range(NCORES):
        b = c // (NCORES // B)
        q0 = (c % (NCORES // B)) * QPC
        out[b, q0 : q0 + QPC] = res.results[c]["out"].astype(np.float32)
    return out
